# revision 17
# baseline (speedup 1.0000x reference)
"""Biaffine labeler kernel for 8 Trainium2 NeuronCores.

Computation (full shapes):
    dep  [2, 2048, 1024], head [2, 2049, 1024], head_indices [2, 2048]
    dep_label  = dep @ dep_W.T + dep_b                    [2, 2048, 512]
    selected   = (head gathered at head_indices) @ head_W.T + head_b
    logits[b,t,n] = dep_label[b,t,:] @ W[n] @ selected[b,t,:] + bias[n]

Sharding: data-parallel over (b, t): core c handles b = c // 4 and the
512-token range starting at (c % 4) * 512.  W / projections replicated.
The head shard each core receives is the 512 rows its tokens select
(the gather is resolved on the host as part of sharding), so no
on-device gather — and therefore no gpsimd SWDGE library — is needed.

Per-core device program (matmuls in bf16, fp32 PSUM accumulation).
The kernel is PE-roofline-bound (800 biaffine matmuls of 128x128x512 =
171us at 2.4GHz bf16); the schedule aims to keep the PE stream airtight:
    1. ones_r memset on gpsimd (up ~1.3us before the vector engine), so
       the PE warmup chain (5 dataless matmuls that ramp the PE out of
       its low-power pstate) starts as soon as the tensor sequencer is
       live (~7.2us) instead of waiting for a vector memset
    2. dep / depW arrive as quarter-size DMAs interleaved across the two
       HWDGE trigger queues (sync + scalar); the dep projection loops
       contraction-outer so phase k needs only quarter k — first real
       matmul right when the warmup chain ends (~9us)
    3. projections on PE; dep bias folds into the ACT psum->sbuf cast
       (per-partition bias AP), head bias via a K=1 matmul
    4. per label pair: W arrives host-pre-cast bf16 in device tile
       layout (1MB DMAs, 8KB per-partition rows, alternating HWDGE
       queues); A_n = dep_label @ W[n] on PE (4 K-chunks x 4 token
       chunks, N=512), one fused DVE scalar_tensor_tensor per chunk
       does logits[:, n] = sum_e A_n * selected (multiply + free-dim
       accumulate)
    5. logits += bias (broadcast via ones x biasn matmul), DMA out
"""

import sys

for _p in ("/opt/trn_rl_repo", "/root/.axon_site/_ro/trn_rl_repo"):
    if _p not in sys.path:
        sys.path.append(_p)

from contextlib import ExitStack

import ml_dtypes
import numpy as np

BF16NP = ml_dtypes.bfloat16

import concourse.bass as bass  # noqa: F401
import concourse.mybir as mybir
import concourse.tile as tile
from concourse import bacc
from concourse.bass_utils import run_bass_kernel_spmd

B, T, D = 2, 2048, 1024
E = 512            # label-space dim (D // 2)
NLAB = 50
NCORES = 8
TLOC = (B * T) // NCORES   # 512 tokens per core
TP = TLOC // 128           # 4 token chunks
DP = D // 128              # 8 contraction chunks for the projections
EP = E // 128              # 4 chunks of the label dim
NQ = 4                     # dep/depW arrive as NQ quarter-DMAs
JQ = DP // NQ              # contraction chunks per quarter

F32 = mybir.dt.float32
BF16 = mybir.dt.bfloat16


def build_program():
    nc = bacc.Bacc("TRN2", target_bir_lowering=False, debug=False,
                   num_devices=NCORES)

    # dep/depW arrive as quarter pieces: small enough that the first
    # projection phase starts ~10.2us, big enough (2KB descriptors) to
    # keep the DMA rings near their peak rate — the whole first ~25us
    # is delivery-bound on the two HWDGE rings (~155GB/s each)
    PIECES = [(0, 2), (2, 2), (4, 2), (6, 2)]   # (j0, width)
    dep_q = [nc.dram_tensor(f"dep_q{k}", [128, w, TLOC], BF16,
                            kind="ExternalInput").ap()
             for k, (_, w) in enumerate(PIECES)]
    depW_q = [nc.dram_tensor(f"depW_q{k}", [128, w, E], BF16,
                             kind="ExternalInput").ap()
              for k, (_, w) in enumerate(PIECES)]
    # sel / headW still ride as halves: they are needed later, after the
    # dep projection has the PE busy for ~7us
    HDP = DP // 2
    sel_A = nc.dram_tensor("sel_A", [128, HDP, TLOC], BF16,
                           kind="ExternalInput").ap()
    sel_B = nc.dram_tensor("sel_B", [128, HDP, TLOC], BF16,
                           kind="ExternalInput").ap()
    headW_A = nc.dram_tensor("headW_A", [128, HDP, E], BF16,
                             kind="ExternalInput").ap()
    headW_B = nc.dram_tensor("headW_B", [128, HDP, E], BF16,
                             kind="ExternalInput").ap()
    depb_c = nc.dram_tensor("depb_c", [128, EP], F32,
                            kind="ExternalInput").ap()
    headb = nc.dram_tensor("headb", [1, E], F32, kind="ExternalInput").ap()
    # host-pre-cast bf16 W in device tile layout: [n, p, d-chunk, e]
    Wb = nc.dram_tensor("Wb", [NLAB, 128, EP, E], BF16,
                        kind="ExternalInput").ap()
    biasn = nc.dram_tensor("biasn", [1, NLAB], F32, kind="ExternalInput").ap()
    logits = nc.dram_tensor("logits", [TLOC, NLAB], F32,
                            kind="ExternalOutput").ap()

    with tile.TileContext(nc) as tc, ExitStack() as ctx:
        # ---- persistent tiles (one pool, one slot per distinct tag) ----
        pp = ctx.enter_context(tc.tile_pool(name="persist", bufs=1))

        def ptile(shape, dtype, name):
            return pp.tile(shape, dtype, tag=name, name=name)

        warm_r = ptile([1, 128], BF16, "warm_r")
        ones_r = ptile([1, TLOC], BF16, "ones_r")
        stage_b = ptile([1, E], F32, "stage_b")
        depb_sb = ptile([128, EP], F32, "depb_sb")
        headb_sb = ptile([1, E], BF16, "headb_sb")
        biasn_f32 = ptile([1, NLAB], F32, "biasn_f32")
        biasn_sb = ptile([1, NLAB], BF16, "biasn_sb")
        bias_bc = ptile([128, NLAB], F32, "bias_bc")
        logit_out = ptile([128, TP, NLAB], F32, "logit_out")
        dep_lT = ptile([128, EP, TLOC], BF16, "dep_lT")   # [e, tok]
        sel_sb = ptile([128, TP, E], BF16, "sel_sb")      # [tok, e]
        dep_s = [ptile([128, w, TLOC], BF16, f"dep_s{k}")
                 for k, (_, w) in enumerate(PIECES)]
        depW_s = [ptile([128, w, E], BF16, f"depW_s{k}")
                  for k, (_, w) in enumerate(PIECES)]
        HDP2 = DP // 2
        sel_rA = ptile([128, HDP2, TLOC], BF16, "sel_rA")
        sel_rB = ptile([128, HDP2, TLOC], BF16, "sel_rB")
        headW_sA = ptile([128, HDP2, E], BF16, "headW_sA")
        headW_sB = ptile([128, HDP2, E], BF16, "headW_sB")
        logit_sb = ptile([128, TP, NLAB], F32, "logit_sb")

        w_pool = ctx.enter_context(tc.tile_pool(name="wn", bufs=4))
        dead_pool = ctx.enter_context(tc.tile_pool(name="dead", bufs=2))

        # warm_r on gpsimd: the pool engine's user instructions start with
        # everyone else's (~7.2us) but this tiny memset retires in ~0.2us,
        # so the PE warmup chain is live by ~7.4us instead of waiting for
        # the big vector-engine memset; ones_r (used only by the bias
        # matmuls ~15us later) goes to the idle vector engine
        nc.gpsimd.memset(warm_r[:], 1.0)
        nc.vector.memset(ones_r[:], 1.0)

        # startup-critical loads, interleaved across the two HWDGE
        # trigger queues so projection phase k's pair of pieces lands
        # ~0.6us after phase k-1's; depb rides early (it gates the ACT
        # psum->sbuf casts that free projection PSUM banks)
        nc.sync.dma_start(dep_s[0][:], dep_q[0])
        nc.scalar.dma_start(depW_s[0][:], depW_q[0])
        nc.sync.dma_start(depW_s[1][:], depW_q[1])
        nc.scalar.dma_start(dep_s[1][:], dep_q[1])
        nc.sync.dma_start(depb_sb[:], depb_c)
        nc.scalar.dma_start(depW_s[2][:], depW_q[2])
        nc.sync.dma_start(dep_s[2][:], dep_q[2])
        nc.scalar.dma_start(dep_s[3][:], dep_q[3])
        nc.sync.dma_start(depW_s[3][:], depW_q[3])

        # all 8 PSUM banks: decouples head-projection PSUM slots from the
        # dep-projection ACT casts that recycle the dep slots
        ps_pool = ctx.enter_context(
            tc.tile_pool(name="ps", bufs=8, space="PSUM"))

        # PE warmup while the dep DMAs land: the PE clock steps to full
        # speed after ~35 matmul instructions (regardless of size), and
        # an idle gap drops it back down.  35 N=96 matmuls (~80ns each
        # at the mid pstate) finish the ramp right as the first dep
        # piece lands (~10.1us), so real work runs at full clock with a
        # gapless fine-grained handoff.
        for _ in range(35):
            psw = ps_pool.tile([128, 512], F32, tag="ps")
            nc.tensor.matmul(psw[:96, :96], warm_r[:1, :96], warm_r[:1, :96],
                             start=True, stop=True)
        for _ in range(6):
            psw = ps_pool.tile([128, 512], F32, tag="ps")
            nc.tensor.matmul(psw[:64, :64], warm_r[:1, :64], warm_r[:1, :64],
                             start=True, stop=True)

        # loads needed later: head projection operands (phase A pair
        # first — the PE reaches them ~2.5us before the phase B pair),
        # then the biases
        nc.scalar.dma_start(headW_sA[:], headW_A)
        nc.sync.dma_start(sel_rA[:], sel_A)
        nc.scalar.dma_start(sel_rB[:], sel_B)
        nc.sync.dma_start(headW_sB[:], headW_B)
        nc.scalar.dma_start(stage_b[:], headb)
        nc.sync.dma_start(biasn_f32[:], biasn)

        # dep projection -> dep_labelT [e, tok], contraction-outer over
        # the data pieces so phase k starts as soon as piece k has
        # landed; dep bias folds into the ACT psum->sbuf cast
        dpsp = [ps_pool.tile([128, 512], F32, tag="ps", name=f"dpsp{i}")
                for i in range(EP)]
        NP = len(PIECES)
        for k, (_, w) in enumerate(PIECES):
            for jj in range(w):
                for i in range(EP):
                    nc.tensor.matmul(dpsp[i][:],
                                     depW_s[k][:, jj, i * 128:(i + 1) * 128],
                                     dep_s[k][:, jj, :],
                                     start=(k == 0 and jj == 0),
                                     stop=(k == NP - 1 and jj == w - 1))
        for i in range(EP):
            nc.scalar.activation(dep_lT[:, i, :], dpsp[i][:],
                                 mybir.ActivationFunctionType.Identity,
                                 bias=depb_sb[:, i:i + 1])

        # bias staging copies AFTER the casts: the ACT engine runs in
        # order, and these depend on late DMAs — ahead of the casts they
        # would stall the cast chain (and with it the PSUM recycling)
        nc.scalar.copy(headb_sb[:], stage_b[:])
        nc.scalar.copy(biasn_sb[:], biasn_f32[:])

        # head projection of pre-gathered rows -> selected [tok, e],
        # phased over the two halves; head bias via a K=1 matmul at
        # group end
        hpsp = []
        for i in range(TP):
            psp = ps_pool.tile([128, 512], F32, tag="ps")
            hpsp.append(psp)
            for j in range(HDP2):
                nc.tensor.matmul(psp[:],
                                 sel_rA[:, j, i * 128:(i + 1) * 128],
                                 headW_sA[:, j, :],
                                 start=(j == 0), stop=False)
        for i in range(TP):
            psp = hpsp[i]
            for j in range(HDP2):
                nc.tensor.matmul(psp[:],
                                 sel_rB[:, j, i * 128:(i + 1) * 128],
                                 headW_sB[:, j, :],
                                 start=False, stop=False)
            nc.tensor.matmul(psp[:], ones_r[:, :128], headb_sb[:],
                             start=False, stop=True)
            nc.scalar.copy(sel_sb[:, i, :], psp[:])

        # bias[n] broadcast across partitions (needed only at the end):
        # ones[128] x biasn
        psb = ps_pool.tile([128, 512], F32, tag="ps")
        nc.tensor.matmul(psb[:, :NLAB], ones_r[:, :128], biasn_sb[:],
                         start=True, stop=True)
        nc.scalar.copy(bias_bc[:], psb[:, :NLAB])

        # biaffine main loop: per-token-chunk PSUM tiles (fine pipelining).
        # Each 1MB W label is split across BOTH HWDGE rings (partition
        # halves keep the 4KB descriptors) so label n lands in ~3.3us of
        # dual-ring time — just ahead of the PE's 3.46us/label consumption
        for n in range(NLAB):
            wt = w_pool.tile([128, EP, E], BF16, tag="wn")
            nc.sync.dma_start(wt[:64], Wb[n][:64])
            nc.scalar.dma_start(wt[64:], Wb[n][64:])
            for i in range(TP):
                psa = ps_pool.tile([128, 512], F32, tag="ps")
                for j in range(EP):
                    nc.tensor.matmul(psa[:],
                                     dep_lT[:, j, i * 128:(i + 1) * 128],
                                     wt[:, j, :],
                                     start=(j == 0), stop=(j == EP - 1))
                dead = dead_pool.tile([128, E], BF16, tag="dead")
                nc.vector.scalar_tensor_tensor(
                    out=dead[:], in0=psa[:], scalar=1.0,
                    in1=sel_sb[:, i, :],
                    op0=mybir.AluOpType.mult, op1=mybir.AluOpType.mult,
                    accum_out=logit_sb[:, i, n:n + 1])

        # per-chunk bias add + store, so each chunk ships as soon as its
        # last label finishes instead of waiting for the whole tensor
        logits_r = logits.rearrange("(i p) n -> p i n", p=128)
        for i in range(TP):
            nc.vector.tensor_add(logit_out[:, i, :], logit_sb[:, i, :],
                                 bias_bc[:])
            nc.sync.dma_start(logits_r[:, i, :], logit_out[:, i, :])

    nc.compile()
    return nc


_NC_CACHE = []


def _get_program():
    if not _NC_CACHE:
        _NC_CACHE.append(build_program())
    return _NC_CACHE[0]


def make_in_maps(dep, head, head_indices, dep_W, dep_b, head_W, head_b, W,
                 bias):
    dep = np.asarray(dep, dtype=np.float32)
    head = np.asarray(head, dtype=np.float32)
    idx = np.asarray(head_indices)
    def dev_layout(a):
        # [x, 1024] operand -> transposed bf16 tile layout [128, 8, x]
        at = np.asarray(a, dtype=np.float32).T.astype(BF16NP)
        return np.ascontiguousarray(
            at.reshape(DP, 128, at.shape[1]).transpose(1, 0, 2))

    # W -> bf16 device tile layout [n, p, j, e] with d = j*128 + p
    Wb = np.ascontiguousarray(
        np.asarray(W, dtype=np.float32).astype(BF16NP)
        .reshape(NLAB, EP, 128, E).transpose(0, 2, 1, 3))

    PIECES = [(0, 2), (2, 2), (4, 2), (6, 2)]

    def pieces(a):
        return [np.ascontiguousarray(a[:, j0:j0 + w]) for j0, w in PIECES]

    def halves(a):
        h = DP // 2
        return (np.ascontiguousarray(a[:, :h]),
                np.ascontiguousarray(a[:, h:]))

    depW_qs = pieces(dev_layout(dep_W))
    headW_A, headW_B = halves(dev_layout(head_W))
    shared = {
        **{f"depW_q{k}": depW_qs[k] for k in range(len(PIECES))},
        "headW_A": headW_A, "headW_B": headW_B,
        # dep bias as per-partition columns: depb_c[p, i] = dep_b[i*128+p]
        "depb_c": np.ascontiguousarray(
            np.asarray(dep_b, dtype=np.float32).reshape(EP, 128).T),
        "headb": np.ascontiguousarray(head_b, dtype=np.float32).reshape(1, E),
        "Wb": Wb,
        "biasn": np.ascontiguousarray(bias, dtype=np.float32).reshape(1, NLAB),
    }
    in_maps = []
    cores_per_b = NCORES // B
    for c in range(NCORES):
        b = c // cores_per_b
        t0 = (c % cores_per_b) * TLOC
        dep_qs = pieces(dev_layout(dep[b, t0:t0 + TLOC]))
        # head shard for this core = the rows its tokens select
        sel_A, sel_B = halves(dev_layout(head[b][idx[b, t0:t0 + TLOC]]))
        in_maps.append({
            **{f"dep_q{k}": dep_qs[k] for k in range(len(PIECES))},
            "sel_A": sel_A, "sel_B": sel_B,
            **shared,
        })
    return in_maps


def run_sharded(inputs, trace=False):
    """Run the SPMD kernel; returns (full_logits, BassKernelResults)."""
    nc = _get_program()
    in_maps = make_in_maps(
        inputs["dep"], inputs["head"], inputs["head_indices"],
        inputs["dep_W"], inputs["dep_b"], inputs["head_W"],
        inputs["head_b"], inputs["W"], inputs["bias"])
    last_err = None
    for attempt in range(3):
        try:
            res = run_bass_kernel_spmd(nc, in_maps, list(range(NCORES)),
                                       trace=trace)
            break
        except Exception as e:  # transient NRT_EXEC device errors
            last_err = e
            if attempt == 2:
                raise
            import time
            time.sleep(5)
    out = np.empty((B, T, NLAB), dtype=np.float32)
    cores_per_b = NCORES // B
    for c in range(NCORES):
        b = c // cores_per_b
        t0 = (c % cores_per_b) * TLOC
        out[b, t0:t0 + TLOC] = res.results[c]["logits"]
    return out, res


def kernel(dep, head, head_indices, mask, dep_W, dep_b, head_W, head_b, W,
           bias):
    out, _ = run_sharded({
        "dep": dep, "head": head, "head_indices": head_indices,
        "dep_W": dep_W, "dep_b": dep_b, "head_W": head_W,
        "head_b": head_b, "W": W, "bias": bias,
    })
    return out


# revision 23
# speedup vs baseline: 1.0129x; 1.0129x over previous
"""Biaffine labeler kernel for 8 Trainium2 NeuronCores.

Computation (full shapes):
    dep  [2, 2048, 1024], head [2, 2049, 1024], head_indices [2, 2048]
    dep_label  = dep @ dep_W.T + dep_b                    [2, 2048, 512]
    selected   = (head gathered at head_indices) @ head_W.T + head_b
    logits[b,t,n] = dep_label[b,t,:] @ W[n] @ selected[b,t,:] + bias[n]

Sharding: data-parallel over (b, t): core c handles b = c // 4 and the
512-token range starting at (c % 4) * 512.  W / projections replicated.
The head shard each core receives is the 512 rows its tokens select
(the gather is resolved on the host as part of sharding), so no
on-device gather — and therefore no gpsimd SWDGE library — is needed.

Per-core device program (matmuls in bf16, fp32 PSUM accumulation).
The kernel is PE-roofline-bound (800 biaffine matmuls of 128x128x512 =
171us at 2.4GHz bf16); the schedule aims to keep the PE stream airtight:
    1. ones_r memset on gpsimd (up ~1.3us before the vector engine), so
       the PE warmup chain (5 dataless matmuls that ramp the PE out of
       its low-power pstate) starts as soon as the tensor sequencer is
       live (~7.2us) instead of waiting for a vector memset
    2. dep / depW arrive as quarter-size DMAs interleaved across the two
       HWDGE trigger queues (sync + scalar); the dep projection loops
       contraction-outer so phase k needs only quarter k — first real
       matmul right when the warmup chain ends (~9us)
    3. projections on PE; dep bias folds into the ACT psum->sbuf cast
       (per-partition bias AP), head bias via a K=1 matmul
    4. per label pair: W arrives host-pre-cast bf16 in device tile
       layout (1MB DMAs, 8KB per-partition rows, alternating HWDGE
       queues); A_n = dep_label @ W[n] on PE (4 K-chunks x 4 token
       chunks, N=512), one fused DVE scalar_tensor_tensor per chunk
       does logits[:, n] = sum_e A_n * selected (multiply + free-dim
       accumulate)
    5. logits += bias (broadcast via ones x biasn matmul), DMA out
"""

import sys

for _p in ("/opt/trn_rl_repo", "/root/.axon_site/_ro/trn_rl_repo"):
    if _p not in sys.path:
        sys.path.append(_p)

from contextlib import ExitStack

import ml_dtypes
import numpy as np

BF16NP = ml_dtypes.bfloat16

import concourse.bass as bass  # noqa: F401
import concourse.mybir as mybir
import concourse.tile as tile
from concourse import bacc
from concourse.bass_utils import run_bass_kernel_spmd

B, T, D = 2, 2048, 1024
E = 512            # label-space dim (D // 2)
NLAB = 50
NCORES = 8
TLOC = (B * T) // NCORES   # 512 tokens per core
TP = TLOC // 128           # 4 token chunks
DP = D // 128              # 8 contraction chunks for the projections
EP = E // 128              # 4 chunks of the label dim
NQ = 4                     # dep/depW arrive as NQ quarter-DMAs
JQ = DP // NQ              # contraction chunks per quarter

F32 = mybir.dt.float32
BF16 = mybir.dt.bfloat16


def build_program():
    nc = bacc.Bacc("TRN2", target_bir_lowering=False, debug=False,
                   num_devices=NCORES)

    # dep/depW arrive as 5 pieces: two eighths (j=0, j=1) for minimum
    # first-data latency, then three quarters.  The whole first ~22us is
    # delivery-bound on the two HWDGE rings, so piece sizing/order is
    # tuned to keep the (ramping) PE fed with zero long gaps.
    PIECES = [(0, 1), (1, 1), (2, 2), (4, 2), (6, 2)]   # (j0, width)
    dep_q = [nc.dram_tensor(f"dep_q{k}", [128, w, TLOC], BF16,
                            kind="ExternalInput").ap()
             for k, (_, w) in enumerate(PIECES)]
    depW_q = [nc.dram_tensor(f"depW_q{k}", [128, w, E], BF16,
                             kind="ExternalInput").ap()
              for k, (_, w) in enumerate(PIECES)]
    # sel / headW still ride as halves: they are needed later, after the
    # dep projection has the PE busy for ~7us
    HDP = DP // 2
    sel_A = nc.dram_tensor("sel_A", [128, HDP, TLOC], BF16,
                           kind="ExternalInput").ap()
    sel_B = nc.dram_tensor("sel_B", [128, HDP, TLOC], BF16,
                           kind="ExternalInput").ap()
    headW_A = nc.dram_tensor("headW_A", [128, HDP, E], BF16,
                             kind="ExternalInput").ap()
    headW_B = nc.dram_tensor("headW_B", [128, HDP, E], BF16,
                             kind="ExternalInput").ap()
    depb_c = nc.dram_tensor("depb_c", [128, EP], F32,
                            kind="ExternalInput").ap()
    headb = nc.dram_tensor("headb", [1, E], F32, kind="ExternalInput").ap()
    # host-pre-cast bf16 W in device tile layout: [n, p, d-chunk, e]
    Wb = nc.dram_tensor("Wb", [NLAB, 128, EP, E], BF16,
                        kind="ExternalInput").ap()
    biasn = nc.dram_tensor("biasn", [1, NLAB], F32, kind="ExternalInput").ap()
    logits = nc.dram_tensor("logits", [TLOC, NLAB], F32,
                            kind="ExternalOutput").ap()

    with tile.TileContext(nc) as tc, ExitStack() as ctx:
        # ---- persistent tiles (one pool, one slot per distinct tag) ----
        pp = ctx.enter_context(tc.tile_pool(name="persist", bufs=1))

        def ptile(shape, dtype, name):
            return pp.tile(shape, dtype, tag=name, name=name)

        warm_r = ptile([1, 128], BF16, "warm_r")
        ones_r = ptile([1, TLOC], BF16, "ones_r")
        stage_b = ptile([1, E], F32, "stage_b")
        depb_sb = ptile([128, EP], F32, "depb_sb")
        headb_sb = ptile([1, E], BF16, "headb_sb")
        biasn_f32 = ptile([1, NLAB], F32, "biasn_f32")
        biasn_sb = ptile([1, NLAB], BF16, "biasn_sb")
        bias_bc = ptile([128, NLAB], F32, "bias_bc")
        logit_out = ptile([128, TP, NLAB], F32, "logit_out")
        dep_lT = ptile([128, EP, TLOC], BF16, "dep_lT")   # [e, tok]
        sel_sb = ptile([128, TP, E], BF16, "sel_sb")      # [tok, e]
        dep_s = [ptile([128, w, TLOC], BF16, f"dep_s{k}")
                 for k, (_, w) in enumerate(PIECES)]
        depW_s = [ptile([128, w, E], BF16, f"depW_s{k}")
                  for k, (_, w) in enumerate(PIECES)]
        HDP2 = DP // 2
        sel_rA = ptile([128, HDP2, TLOC], BF16, "sel_rA")
        sel_rB = ptile([128, HDP2, TLOC], BF16, "sel_rB")
        headW_sA = ptile([128, HDP2, E], BF16, "headW_sA")
        headW_sB = ptile([128, HDP2, E], BF16, "headW_sB")
        logit_sb = ptile([128, TP, NLAB], F32, "logit_sb")

        w_pool = ctx.enter_context(tc.tile_pool(name="wn", bufs=4))
        dead_pool = ctx.enter_context(tc.tile_pool(name="dead", bufs=2))

        # warm_r on gpsimd: the pool engine's user instructions start with
        # everyone else's (~7.2us) but this tiny memset retires in ~0.2us,
        # so the PE warmup chain is live by ~7.4us instead of waiting for
        # the big vector-engine memset; ones_r (used only by the bias
        # matmuls ~15us later) goes to the idle vector engine
        nc.gpsimd.memset(warm_r[:], 1.0)
        nc.vector.memset(ones_r[:], 1.0)

        # startup-critical loads, interleaved across the two HWDGE
        # trigger queues so projection phase k's pair of pieces lands
        # ~0.6us after phase k-1's; depb rides early (it gates the ACT
        # psum->sbuf casts that free projection PSUM banks)
        nc.sync.dma_start(dep_s[0][:], dep_q[0])
        nc.scalar.dma_start(depW_s[0][:], depW_q[0])
        nc.sync.dma_start(depW_s[1][:], depW_q[1])
        nc.scalar.dma_start(dep_s[1][:], dep_q[1])
        nc.sync.dma_start(depb_sb[:], depb_c)
        nc.scalar.dma_start(depW_s[2][:], depW_q[2])
        nc.sync.dma_start(dep_s[2][:], dep_q[2])
        nc.scalar.dma_start(dep_s[3][:], dep_q[3])
        nc.sync.dma_start(depW_s[3][:], depW_q[3])
        nc.scalar.dma_start(depW_s[4][:], depW_q[4])
        nc.sync.dma_start(dep_s[4][:], dep_q[4])

        # all 8 PSUM banks: decouples head-projection PSUM slots from the
        # dep-projection ACT casts that recycle the dep slots
        ps_pool = ctx.enter_context(
            tc.tile_pool(name="ps", bufs=8, space="PSUM"))

        # PE warmup while the dep DMAs land: the PE clock ramps per
        # retired matmul (~35 instructions to full speed) and an idle
        # gap drops it back down with a costly ~35-instruction re-ramp.
        # The first ~22us is DMA-delivery-bound, so the PE *should*
        # cruise at the mid pstate through the projections — N=128
        # warmups (107ns) pace the ramp so full clock arrives roughly
        # when the delivery-bound phase ends, with no long PE gap
        # anywhere (measured best across warmup variants).
        for _ in range(26):
            psw = ps_pool.tile([128, 512], F32, tag="ps")
            nc.tensor.matmul(psw[:, :128], warm_r[:1, :], warm_r[:1, :],
                             start=True, stop=True)
        for _ in range(6):
            psw = ps_pool.tile([128, 512], F32, tag="ps")
            nc.tensor.matmul(psw[:64, :64], warm_r[:1, :64], warm_r[:1, :64],
                             start=True, stop=True)

        # loads needed later: head projection operands (phase A pair
        # first — the PE reaches them ~2.5us before the phase B pair),
        # then the biases
        nc.scalar.dma_start(headW_sA[:], headW_A)
        nc.sync.dma_start(sel_rA[:], sel_A)
        nc.scalar.dma_start(sel_rB[:], sel_B)
        nc.sync.dma_start(headW_sB[:], headW_B)
        nc.scalar.dma_start(stage_b[:], headb)
        nc.sync.dma_start(biasn_f32[:], biasn)

        # dep projection -> dep_labelT [e, tok], contraction-outer over
        # the data pieces so phase k starts as soon as piece k has
        # landed; dep bias folds into the ACT psum->sbuf cast
        dpsp = [ps_pool.tile([128, 512], F32, tag="ps", name=f"dpsp{i}")
                for i in range(EP)]
        NP = len(PIECES)
        for k, (_, w) in enumerate(PIECES):
            for jj in range(w):
                for i in range(EP):
                    nc.tensor.matmul(dpsp[i][:],
                                     depW_s[k][:, jj, i * 128:(i + 1) * 128],
                                     dep_s[k][:, jj, :],
                                     start=(k == 0 and jj == 0),
                                     stop=(k == NP - 1 and jj == w - 1))
        for i in range(EP):
            nc.scalar.activation(dep_lT[:, i, :], dpsp[i][:],
                                 mybir.ActivationFunctionType.Identity,
                                 bias=depb_sb[:, i:i + 1])

        # bias staging copies AFTER the casts: the ACT engine runs in
        # order, and these depend on late DMAs — ahead of the casts they
        # would stall the cast chain (and with it the PSUM recycling)
        nc.scalar.copy(headb_sb[:], stage_b[:])
        nc.scalar.copy(biasn_sb[:], biasn_f32[:])

        # keep-alive filler before the head projection: sel/headW land
        # within ~0.5us of the PE getting here; tiny matmuls absorb that
        # jitter without an idle gap (which would drop the PE clock)
        for _ in range(10):
            psw = ps_pool.tile([128, 512], F32, tag="ps")
            nc.tensor.matmul(psw[:64, :64], warm_r[:1, :64], warm_r[:1, :64],
                             start=True, stop=True)

        # head projection of pre-gathered rows -> selected [tok, e],
        # phased over the two halves; head bias via a K=1 matmul at
        # group end
        hpsp = []
        for i in range(TP):
            psp = ps_pool.tile([128, 512], F32, tag="ps")
            hpsp.append(psp)
            for j in range(HDP2):
                nc.tensor.matmul(psp[:],
                                 sel_rA[:, j, i * 128:(i + 1) * 128],
                                 headW_sA[:, j, :],
                                 start=(j == 0), stop=False)
        for i in range(TP):
            psp = hpsp[i]
            for j in range(HDP2):
                nc.tensor.matmul(psp[:],
                                 sel_rB[:, j, i * 128:(i + 1) * 128],
                                 headW_sB[:, j, :],
                                 start=False, stop=False)
            nc.tensor.matmul(psp[:], ones_r[:, :128], headb_sb[:],
                             start=False, stop=True)
            nc.scalar.copy(sel_sb[:, i, :], psp[:])

        # bias[n] broadcast across partitions (needed only at the end):
        # ones[128] x biasn
        psb = ps_pool.tile([128, 512], F32, tag="ps")
        nc.tensor.matmul(psb[:, :NLAB], ones_r[:, :128], biasn_sb[:],
                         start=True, stop=True)
        nc.scalar.copy(bias_bc[:], psb[:, :NLAB])

        # biaffine main loop: per-token-chunk PSUM tiles (fine pipelining)
        for n in range(NLAB):
            wt = w_pool.tile([128, EP, E], BF16, tag="wn")
            eng = nc.sync if n % 2 == 0 else nc.scalar
            eng.dma_start(wt[:], Wb[n])
            for i in range(TP):
                psa = ps_pool.tile([128, 512], F32, tag="ps")
                for j in range(EP):
                    nc.tensor.matmul(psa[:],
                                     dep_lT[:, j, i * 128:(i + 1) * 128],
                                     wt[:, j, :],
                                     start=(j == 0), stop=(j == EP - 1))
                dead = dead_pool.tile([128, E], BF16, tag="dead")
                nc.vector.scalar_tensor_tensor(
                    out=dead[:], in0=psa[:], scalar=1.0,
                    in1=sel_sb[:, i, :],
                    op0=mybir.AluOpType.mult, op1=mybir.AluOpType.mult,
                    accum_out=logit_sb[:, i, n:n + 1])

        # per-chunk bias add + store, so each chunk ships as soon as its
        # last label finishes instead of waiting for the whole tensor
        logits_r = logits.rearrange("(i p) n -> p i n", p=128)
        for i in range(TP):
            nc.vector.tensor_add(logit_out[:, i, :], logit_sb[:, i, :],
                                 bias_bc[:])
            nc.sync.dma_start(logits_r[:, i, :], logit_out[:, i, :])

    nc.compile()
    return nc


_NC_CACHE = []


def _get_program():
    if not _NC_CACHE:
        _NC_CACHE.append(build_program())
    return _NC_CACHE[0]


def make_in_maps(dep, head, head_indices, dep_W, dep_b, head_W, head_b, W,
                 bias):
    dep = np.asarray(dep, dtype=np.float32)
    head = np.asarray(head, dtype=np.float32)
    idx = np.asarray(head_indices)
    def dev_layout(a):
        # [x, 1024] operand -> transposed bf16 tile layout [128, 8, x]
        at = np.asarray(a, dtype=np.float32).T.astype(BF16NP)
        return np.ascontiguousarray(
            at.reshape(DP, 128, at.shape[1]).transpose(1, 0, 2))

    # W -> bf16 device tile layout [n, p, j, e] with d = j*128 + p
    Wb = np.ascontiguousarray(
        np.asarray(W, dtype=np.float32).astype(BF16NP)
        .reshape(NLAB, EP, 128, E).transpose(0, 2, 1, 3))

    PIECES = [(0, 1), (1, 1), (2, 2), (4, 2), (6, 2)]

    def pieces(a):
        return [np.ascontiguousarray(a[:, j0:j0 + w]) for j0, w in PIECES]

    def halves(a):
        h = DP // 2
        return (np.ascontiguousarray(a[:, :h]),
                np.ascontiguousarray(a[:, h:]))

    depW_qs = pieces(dev_layout(dep_W))
    headW_A, headW_B = halves(dev_layout(head_W))
    shared = {
        **{f"depW_q{k}": depW_qs[k] for k in range(len(PIECES))},
        "headW_A": headW_A, "headW_B": headW_B,
        # dep bias as per-partition columns: depb_c[p, i] = dep_b[i*128+p]
        "depb_c": np.ascontiguousarray(
            np.asarray(dep_b, dtype=np.float32).reshape(EP, 128).T),
        "headb": np.ascontiguousarray(head_b, dtype=np.float32).reshape(1, E),
        "Wb": Wb,
        "biasn": np.ascontiguousarray(bias, dtype=np.float32).reshape(1, NLAB),
    }
    in_maps = []
    cores_per_b = NCORES // B
    for c in range(NCORES):
        b = c // cores_per_b
        t0 = (c % cores_per_b) * TLOC
        dep_qs = pieces(dev_layout(dep[b, t0:t0 + TLOC]))
        # head shard for this core = the rows its tokens select
        sel_A, sel_B = halves(dev_layout(head[b][idx[b, t0:t0 + TLOC]]))
        in_maps.append({
            **{f"dep_q{k}": dep_qs[k] for k in range(len(PIECES))},
            "sel_A": sel_A, "sel_B": sel_B,
            **shared,
        })
    return in_maps


def run_sharded(inputs, trace=False):
    """Run the SPMD kernel; returns (full_logits, BassKernelResults)."""
    nc = _get_program()
    in_maps = make_in_maps(
        inputs["dep"], inputs["head"], inputs["head_indices"],
        inputs["dep_W"], inputs["dep_b"], inputs["head_W"],
        inputs["head_b"], inputs["W"], inputs["bias"])
    last_err = None
    for attempt in range(3):
        try:
            res = run_bass_kernel_spmd(nc, in_maps, list(range(NCORES)),
                                       trace=trace)
            break
        except Exception as e:  # transient NRT_EXEC device errors
            last_err = e
            if attempt == 2:
                raise
            import time
            time.sleep(5)
    out = np.empty((B, T, NLAB), dtype=np.float32)
    cores_per_b = NCORES // B
    for c in range(NCORES):
        b = c // cores_per_b
        t0 = (c % cores_per_b) * TLOC
        out[b, t0:t0 + TLOC] = res.results[c]["logits"]
    return out, res


def kernel(dep, head, head_indices, mask, dep_W, dep_b, head_W, head_b, W,
           bias):
    out, _ = run_sharded({
        "dep": dep, "head": head, "head_indices": head_indices,
        "dep_W": dep_W, "dep_b": dep_b, "head_W": head_W,
        "head_b": head_b, "W": W, "bias": bias,
    })
    return out
